# revision 4
# baseline (speedup 1.0000x reference)
"""GATv2 GNN message passing on 8 TRN2 NeuronCores — gather-free version.

Strategy (host pre-permuted source features, edge/dst-tiled, no collectives):
  - Host sorts edges by dst, packs tiles of <=128 contiguous dst nodes and
    <=640 edges, splits tiles until count == 8*T, LPT-balances tiles to
    cores by edge count.
  - Host pre-permutes x[src] per edge (7 floats) so the device recomputes
    h_src = relu(W_ne^T x_src) per edge instead of gathering 2KB xl-table
    rows (kills the GpSimd dma_gather and the AllGather prologue).
  - s' = vscale*(x_l[src]+x_r[dst]+e_h) per edge via matmuls (c-major
    operands); leaky via 0.2*lin + relu part; logits via CTTR (DVE).
  - Aggregation in h-space: u[dst,h,:] = sum alpha*h_src (256-wide), then
    decoder uses host-folded W_ld1[h] = W_l[:,h-block] @ W_d1[h-block,:].
  - Isolated (deg-0) nodes fixed up on host with a constant.
  - sigmoid via tanh (exp/sigmoid never share an ACT table).
"""

import os
import numpy as np
import ml_dtypes

bf16 = ml_dtypes.bfloat16

N_NODES = 20000
N_EDGES = 100000
HID = 256
HEADS = 4
OUTC = 256
HC = HEADS * OUTC  # 1024
N_CORES = 8
EDGE_CAP = 640
CHUNKS = EDGE_CAP // 128  # 5
XR_W = HC + HEADS  # 1028

_CACHE = {}


# --------------------------------------------------------------------------
# host-side preprocessing
# --------------------------------------------------------------------------

def _chunk2(w):  # [256, K] -> [128, 2, K]
    return np.ascontiguousarray(w.reshape(2, 128, -1).transpose(1, 0, 2))


def _host_prep(inputs):
    x = np.asarray(inputs["x"], np.float32)
    ea = np.asarray(inputs["edge_attr"], np.float32)
    ei = np.asarray(inputs["edge_index"])
    src = ei[0].astype(np.int64)
    dst = ei[1].astype(np.int64)

    order = np.argsort(dst, kind="stable")
    s_s = src[order]
    d_s = dst[order]
    ea_s = ea[order]

    deg = np.bincount(dst, minlength=N_NODES)
    cum = np.concatenate([[0], np.cumsum(deg)])
    assert deg.max() <= EDGE_CAP, f"node degree {deg.max()} exceeds tile cap"

    # greedy tiles: contiguous nodes, <=128 nodes, <=EDGE_CAP edges
    tiles = []  # (n0, n1)
    n0 = 0
    while n0 < N_NODES:
        n1 = min(n0 + 128, N_NODES)
        while cum[n1] - cum[n0] > EDGE_CAP:
            n1 -= 1
        tiles.append((n0, n1))
        n0 = n1

    # split largest tiles (by edges) until count is a multiple of 8
    def edges_of(t):
        return int(cum[t[1]] - cum[t[0]])

    T = -(-len(tiles) // 8)  # ceil
    while len(tiles) < 8 * T:
        i = max(range(len(tiles)),
                key=lambda k: (edges_of(tiles[k]), tiles[k][1] - tiles[k][0]))
        a0, a1 = tiles[i]
        if a1 - a0 < 2:  # cannot split 1-node tile; split any >=2 instead
            i = max(range(len(tiles)),
                    key=lambda k: tiles[k][1] - tiles[k][0])
            a0, a1 = tiles[i]
        # split node range to balance edges
        best, bm = a0 + 1, None
        for m in range(a0 + 1, a1):
            c = max(cum[m] - cum[a0], cum[a1] - cum[m])
            if bm is None or c < bm:
                bm, best = c, m
        tiles[i:i + 1] = [(a0, best), (best, a1)]

    # LPT: assign T tiles per core balancing edges
    order_t = sorted(range(len(tiles)), key=lambda k: -edges_of(tiles[k]))
    core_tiles = [[] for _ in range(N_CORES)]
    core_load = [0] * N_CORES
    for k in order_t:
        cands = [c for c in range(N_CORES) if len(core_tiles[c]) < T]
        c = min(cands, key=lambda cc: core_load[cc])
        core_tiles[c].append(k)
        core_load[c] += edges_of(tiles[k])

    # weights
    W_ne = np.asarray(inputs["W_ne"], np.float32)
    W_ee = np.asarray(inputs["W_ee"], np.float32)
    W_l = np.asarray(inputs["W_l"], np.float32)
    W_r = np.asarray(inputs["W_r"], np.float32)
    W_e = np.asarray(inputs["W_e"], np.float32)
    att = np.asarray(inputs["att"], np.float32)
    W_d1 = np.asarray(inputs["W_d1"], np.float32)
    W_d2 = np.asarray(inputs["W_d2"], np.float32)
    b_ne = np.asarray(inputs["b_ne"], np.float32)
    b_ee = np.asarray(inputs["b_ee"], np.float32)
    b_l = np.asarray(inputs["b_l"], np.float32)
    b_r = np.asarray(inputs["b_r"], np.float32)
    b_e = np.asarray(inputs["b_e"], np.float32)
    conv_bias = np.asarray(inputs["conv_bias"], np.float32)
    b_d1 = np.asarray(inputs["b_d1"], np.float32)
    b_d2 = np.asarray(inputs["b_d2"], np.float32)

    attf = att.reshape(-1)  # [1024] head-major
    vscale = 0.8 * np.abs(attf)
    sgn = np.where(attf >= 0, 1.0, -1.0).astype(np.float32)
    att_bd = np.zeros((HC, HEADS), np.float32)
    for h in range(HEADS):
        att_bd[h * OUTC:(h + 1) * OUTC, h] = att[h]

    btot = b_l + b_r + b_e
    # xr extended weights: scaled W_r columns + 0.2*att-dot columns
    W_re = np.concatenate([W_r * vscale[None, :], 0.2 * (W_r @ att_bd)], 1)
    btot_row = np.concatenate([btot * vscale, 0.2 * (btot @ att_bd)])

    # decoder fold: W_ld1[h] = W_l_blk @ W_d1_blk; bias absorbs b_l via
    # sum(alpha)=1 (isolated nodes fixed on host)
    w_ld1 = np.zeros((128, 8, HID), np.float32)
    for h in range(HEADS):
        blk = W_l[:, h * OUTC:(h + 1) * OUTC] @ W_d1[h * OUTC:(h + 1) * OUTC]
        for half in range(2):
            w_ld1[:, h * 2 + half, :] = blk[half * 128:(half + 1) * 128, :]
    b_d1p = b_d1 + conv_bias @ W_d1 + b_l @ W_d1

    iso_z1 = np.maximum(conv_bias @ W_d1 + b_d1, 0.0)
    iso_out = 1.0 / (1.0 + np.exp(-(iso_z1 @ W_d2 + b_d2)))  # [6]

    flags = (int(len(tiles) // 8), bool(np.any(btot != 0.0)),
             bool(np.any(b_d2 != 0.0)))

    per_core_common = {
        "w_ne": W_ne.astype(bf16),  # [7, 256]
        "w_ee": W_ee.astype(bf16),
        "w_es": _chunk2(W_e * vscale[None, :]).astype(bf16),  # [128,2,1024]
        "w_ls": _chunk2(W_l * vscale[None, :]).astype(bf16),
        "w_re": _chunk2(W_re).astype(bf16),  # [128, 2, 1028]
        "w_ae": _chunk2(0.2 * (W_e @ att_bd)).astype(bf16),  # [128, 2, 4]
        "w_al": _chunk2(0.2 * (W_l @ att_bd)).astype(bf16),
        "w_ld1": w_ld1.astype(bf16),  # [128, 8, 256]
        "w_d2": _chunk2(W_d2).astype(bf16),  # [128, 2, 6]
        "b_ne": _chunk2(b_ne.reshape(HID, 1)),  # [128, 2, 1] f32
        "b_ee": _chunk2(b_ee.reshape(HID, 1)),
        "b_d1p": _chunk2(b_d1p.reshape(HID, 1)),
        "i128": np.eye(128, dtype=bf16),
        "iota": np.broadcast_to(
            np.arange(128, dtype=np.float32), (128, 128)).copy(),
        "sgn_b": np.broadcast_to(sgn, (128, HC)).astype(bf16).copy(),
        "btot_r": btot_row.reshape(1, XR_W).astype(np.float32),
        "b_d2b": np.broadcast_to(b_d2, (128, 6)).astype(np.float32).copy(),
    }

    in_maps = []
    meta = []
    for c in range(N_CORES):
        ctiles = [tiles[k] for k in core_tiles[c]]
        eaT = np.zeros((T, 7, EDGE_CAP), bf16)
        xsT = np.zeros((T, 7, EDGE_CAP), bf16)
        dstloc = np.zeros((T, 128, CHUNKS), np.float32) - 1.0
        xloc = np.zeros((T * 128, 7), np.float32)
        for t, (a0, a1) in enumerate(ctiles):
            e0, e1 = int(cum[a0]), int(cum[a1])
            ne = e1 - e0
            nv = a1 - a0
            xloc[t * 128:t * 128 + nv] = x[a0:a1]
            if ne == 0:
                continue
            eaT[t, :, :ne] = ea_s[e0:e1].T.astype(bf16)
            xsT[t, :, :ne] = x[s_s[e0:e1]].T.astype(bf16)
            dl = np.full(EDGE_CAP, -1.0, np.float32)
            dl[:ne] = (d_s[e0:e1] - a0).astype(np.float32)
            dstloc[t] = dl.reshape(CHUNKS, 128).T
        m = dict(per_core_common)
        m["eaT"] = eaT
        m["xsT"] = xsT
        m["dstloc"] = dstloc
        m["xlocT"] = np.ascontiguousarray(xloc.T).astype(bf16)  # [7, T*128]
        in_maps.append(m)
        meta.append(ctiles)
    return in_maps, meta, flags, (deg, iso_out)


# --------------------------------------------------------------------------
# bass graph
# --------------------------------------------------------------------------

def _build(flags):
    T, has_btot, has_bd2 = flags
    import concourse.bass as bass
    import concourse.bacc as bacc
    import concourse.mybir as mybir
    import concourse.tile as tile
    from concourse.dve_ops import TENSOR_TENSOR_REDUCE as CTTR

    dt = mybir.dt
    F32, BF16 = dt.float32, dt.bfloat16
    AF = mybir.ActivationFunctionType
    ALU = mybir.AluOpType

    nc = bacc.Bacc("TRN2", target_bir_lowering=False, debug=False,
                   enable_asserts=False, num_devices=N_CORES)

    def din(name, shape, dtype):
        return nc.dram_tensor(name, shape, dtype, kind="ExternalInput")

    w_ne_d = din("w_ne", [7, HID], BF16)
    w_ee_d = din("w_ee", [7, HID], BF16)
    w_es_d = din("w_es", [128, 2, HC], BF16)
    w_ls_d = din("w_ls", [128, 2, HC], BF16)
    w_re_d = din("w_re", [128, 2, XR_W], BF16)
    w_ae_d = din("w_ae", [128, 2, HEADS], BF16)
    w_al_d = din("w_al", [128, 2, HEADS], BF16)
    w_ld1_d = din("w_ld1", [128, 8, HID], BF16)
    w_d2_d = din("w_d2", [128, 2, 6], BF16)
    b_ne_d = din("b_ne", [128, 2, 1], F32)
    b_ee_d = din("b_ee", [128, 2, 1], F32)
    b_d1p_d = din("b_d1p", [128, 2, 1], F32)
    i128_d = din("i128", [128, 128], BF16)
    iota_d = din("iota", [128, 128], F32)
    sgn_b_d = din("sgn_b", [128, HC], BF16)
    btot_r_d = din("btot_r", [1, XR_W], F32)
    b_d2b_d = din("b_d2b", [128, 6], F32)
    eaT_d = din("eaT", [T, 7, EDGE_CAP], BF16)
    xsT_d = din("xsT", [T, 7, EDGE_CAP], BF16)
    dl_d = din("dstloc", [T, 128, CHUNKS], F32)
    xlocT_d = din("xlocT", [7, T * 128], BF16)
    out_d = nc.dram_tensor("out", [T, 128, 6], F32, kind="ExternalOutput")

    with tile.TileContext(nc) as tc:
        with (
            tc.tile_pool(name="const", bufs=1) as cpool,
            tc.tile_pool(name="mn", bufs=2) as mpool,
            tc.tile_pool(name="mn3", bufs=3) as m3pool,
            tc.tile_pool(name="ps_s", bufs=2, space="PSUM") as ps_s_p,
            tc.tile_pool(name="ps_tp", bufs=1, space="PSUM") as ps_tp_p,
            tc.tile_pool(name="ps_ld", bufs=1, space="PSUM") as ps_ld_p,
            tc.tile_pool(name="ps_u", bufs=1, space="PSUM") as ps_u_p,
            tc.tile_pool(name="ps_at", bufs=1, space="PSUM") as ps_at_p,
            tc.tile_pool(name="ps_d1", bufs=1, space="PSUM") as ps_d1_p,
        ):
            # ---- constants in SBUF
            w_ne_s = cpool.tile([7, HID], BF16)
            w_ee_s = cpool.tile([7, HID], BF16)
            w_es_s = cpool.tile([128, 2, HC], BF16)
            w_ls_s = cpool.tile([128, 2, HC], BF16)
            w_re_s = cpool.tile([128, 2, XR_W], BF16)
            w_ae_s = cpool.tile([128, 2, HEADS], BF16)
            w_al_s = cpool.tile([128, 2, HEADS], BF16)
            w_ld1_s = cpool.tile([128, 8, HID], BF16)
            w_d2_s = cpool.tile([128, 2, 6], BF16)
            b_ne_s = cpool.tile([128, 2, 1], F32)
            b_ee_s = cpool.tile([128, 2, 1], F32)
            b_d1p_s = cpool.tile([128, 2, 1], F32)
            i128_s = cpool.tile([128, 128], BF16)
            iota_s = cpool.tile([128, 128], F32)
            sgn_b_s = cpool.tile([128, HC], BF16)
            b_d2b_s = cpool.tile([128, 6], F32)
            xlocT_s = cpool.tile([7, T * 128], BF16)
            hT_loc = cpool.tile([128, 2, T * 128], BF16)
            for dst_t, src_t in [
                (w_ne_s, w_ne_d), (w_ee_s, w_ee_d), (w_es_s, w_es_d),
                (w_ls_s, w_ls_d), (w_re_s, w_re_d), (w_ae_s, w_ae_d),
                (w_al_s, w_al_d), (w_ld1_s, w_ld1_d), (w_d2_s, w_d2_d),
                (b_ne_s, b_ne_d), (b_ee_s, b_ee_d), (b_d1p_s, b_d1p_d),
                (i128_s, i128_d), (iota_s, iota_d), (sgn_b_s, sgn_b_d),
                (b_d2b_s, b_d2b_d), (xlocT_s, xlocT_d),
            ]:
                nc.sync.dma_start(dst_t[:], src_t[:])
            if has_btot:
                ones1_s = cpool.tile([1, 128], BF16)
                btot_rf = cpool.tile([1, XR_W], F32)
                btot_rs = cpool.tile([1, XR_W], BF16)
                nc.vector.memset(ones1_s[:], 1.0)
                nc.sync.dma_start(btot_rf[:], btot_r_d[:])
                nc.vector.tensor_copy(btot_rs[:], btot_rf[:])

            # ---- prologue: hT_loc = relu(W_ne^T @ xlocT)  (c-major)
            i = 0
            for c0 in range(0, T * 128, 512):
                n = min(512, T * 128 - c0)
                for half in range(2):
                    ps = ps_s_p.tile([128, 512], F32, tag="s")
                    nc.tensor.matmul(
                        ps[:, :n], w_ne_s[:, half * 128:(half + 1) * 128],
                        xlocT_s[:, c0:c0 + n], start=True, stop=True)
                    if i % 2 == 0:
                        nc.scalar.activation(
                            hT_loc[:, half, c0:c0 + n], ps[:, :n], AF.Relu,
                            bias=b_ne_s[:, half, 0:1])
                    else:
                        nc.vector.tensor_scalar(
                            hT_loc[:, half, c0:c0 + n], ps[:, :n],
                            b_ne_s[:, half, 0:1], 0.0,
                            op0=ALU.add, op1=ALU.max)
                    i += 1

            def enc(dst_tile, src_dram, t, w_s, b_s, alt):
                """dst[:, half, :640] = relu(w^T @ srcT + b); c-major bf16."""
                st = mpool.tile([7, EDGE_CAP], BF16, tag=f"in{alt}")
                nc.sync.dma_start(st[:], src_dram[t])
                k = 0
                for half in range(2):
                    for c0, n in ((0, 512), (512, 128)):
                        ps = ps_s_p.tile([128, 512], F32, tag="s")
                        nc.tensor.matmul(
                            ps[:, :n], w_s[:, half * 128:(half + 1) * 128],
                            st[:, c0:c0 + n], start=True, stop=True)
                        if (k + alt) % 2 == 0:
                            nc.scalar.activation(
                                dst_tile[:, half, c0:c0 + n], ps[:, :n],
                                AF.Relu, bias=b_s[:, half, 0:1])
                        else:
                            nc.vector.tensor_scalar(
                                dst_tile[:, half, c0:c0 + n], ps[:, :n],
                                b_s[:, half, 0:1], 0.0,
                                op0=ALU.add, op1=ALU.max)
                        k += 1
                return st

            def stage_a(t):
                dl_t = mpool.tile([128, CHUNKS], F32, tag="dl")
                nc.sync.dma_start(dl_t[:], dl_d[t])
                eT = mpool.tile([128, 2, EDGE_CAP], BF16, tag="eT")
                enc(eT, eaT_d, t, w_ee_s, b_ee_s, 0)
                hT = mpool.tile([128, 2, EDGE_CAP], BF16, tag="hT")
                xs_s = enc(hT, xsT_d, t, w_ne_s, b_ne_s, 1)

                # xr rows for this tile's 128 dst nodes: [128, 1028] bf16
                xr = mpool.tile([128, XR_W], BF16, tag="xr")
                hloc = hT_loc[:, :, t * 128:(t + 1) * 128]
                for c0, n, tg in ((0, 512, "s"), (512, 512, "s"),
                                  (1024, HEADS, "ld")):
                    if tg == "s":
                        psx = ps_s_p.tile([128, 512], F32, tag="s",
                                          name="psx_s")
                    else:
                        psx = ps_ld_p.tile([128, 16], F32, tag="ld",
                                           name="psx_ld")
                    for half in range(2):
                        nc.tensor.matmul(
                            psx[:, :n], hloc[:, half, :],
                            w_re_s[:, half, c0:c0 + n],
                            start=(half == 0),
                            stop=(half == 1) and not has_btot)
                    if has_btot:
                        nc.tensor.matmul(
                            psx[:, :n], ones1_s[:, :],
                            btot_rs[:, c0:c0 + n], start=False, stop=True)
                    if c0 == 0:
                        nc.scalar.activation(xr[:, c0:c0 + n], psx[:, :n],
                                             AF.Copy)
                    else:
                        nc.vector.tensor_copy(xr[:, c0:c0 + n], psx[:, :n])

                ps_u = ps_u_p.tile([128, HC], F32, tag="u")
                u_first = {}
                rec = mpool.tile([128, HEADS], F32, tag="rec")
                expb_all = mpool.tile([128, CHUNKS, HEADS], BF16, tag="exb")

                for j in range(CHUNKS):
                    ecols = slice(j * 128, (j + 1) * 128)
                    # one-hot (edge-part x dst-free), bf16
                    eq = m3pool.tile([128, 128], BF16, tag="eq")
                    nc.vector.tensor_scalar(
                        eq[:], iota_s[:], dl_t[:, j:j + 1], None,
                        op0=ALU.is_equal)
                    # transposes: eqT + h_em share one psum bank (one group)
                    tp = ps_tp_p.tile([128, 3, 128], BF16, tag="tp")
                    tp0 = nc.tensor.matmul(
                        tp[:, 0, :], eq[:], i128_s[:], is_transpose=True,
                        start=True, stop=False, skip_group_check=True)
                    for half in range(2):
                        tpm = nc.tensor.matmul(
                            tp[:, 1 + half, :], hT[:, half, ecols],
                            i128_s[:], is_transpose=True,
                            start=False, stop=(half == 1),
                            skip_group_check=True)
                        tile.add_dep_helper(
                            tpm.ins, tp0.ins, reason="psum zero order")
                    eqT = m3pool.tile([128, 128], BF16, tag="eqT")
                    nc.scalar.activation(eqT[:], tp[:, 0, :], AF.Copy)
                    h_em = m3pool.tile([128, 256], BF16, tag="h_em")
                    nc.vector.tensor_copy(
                        h_em[:], tp[:, 1:3, :].rearrange("p a b -> p (a b)"))

                    # lin = 0.2*att.(x_l+x_r+e_h+btot) : [128, 4] psum
                    ps_ld = ps_ld_p.tile([128, 16], F32, tag="ld")
                    lin_mm = nc.tensor.matmul(
                        ps_ld[:, 0:HEADS], eT[:, 0, ecols], w_ae_s[:, 0, :],
                        start=True, stop=False, skip_group_check=True)
                    for half in range(2):
                        nc.tensor.matmul(
                            ps_ld[:, 0:HEADS], hT[:, half, ecols],
                            w_al_s[:, half, :], start=False, stop=False,
                            skip_group_check=True)
                    nc.tensor.matmul(
                        ps_ld[:, 0:HEADS], eT[:, 1, ecols], w_ae_s[:, 1, :],
                        start=False, stop=False, skip_group_check=True)
                    nc.tensor.matmul(
                        ps_ld[:, 0:HEADS], eqT[:], xr[:, HC:HC + HEADS],
                        start=False, stop=True, skip_group_check=True)
                    lin = m3pool.tile([128, HEADS], F32, tag="lin")
                    nc.scalar.activation(lin[:], ps_ld[:, 0:HEADS], AF.Copy)

                    # s' channels: [128, 1024] over two 512-col psum halves
                    zr = m3pool.tile([128, 4, 256], BF16, tag="zr")
                    for k in range(2):
                        kcol = slice(k * 512, (k + 1) * 512)
                        ps = ps_s_p.tile([128, 512], F32, tag="s")
                        for half in range(2):
                            nc.tensor.matmul(
                                ps[:], eT[:, half, ecols],
                                w_es_s[:, half, kcol],
                                start=(half == 0), stop=False)
                        for half in range(2):
                            nc.tensor.matmul(
                                ps[:], hT[:, half, ecols],
                                w_ls_s[:, half, kcol],
                                start=False, stop=False)
                        nc.tensor.matmul(
                            ps[:], eqT[:], xr[:, kcol],
                            start=False, stop=True)
                        nc.scalar.activation(
                            zr[:].rearrange("p a b -> p (a b)")[:, kcol],
                            ps[:], AF.Relu)

                    # logits = lin + sum_c sgn*zr  (CTTR per head)
                    lgj = m3pool.tile([128, HEADS], F32, tag="lgj")
                    z2 = m3pool.tile([128, HC], BF16, tag="z2")
                    for h in range(HEADS):
                        nc.vector._custom_dve(
                            CTTR,
                            out=z2[:, h * OUTC:(h + 1) * OUTC],
                            in0=zr[:, h, :],
                            in1=sgn_b_s[:, h * OUTC:(h + 1) * OUTC],
                            s0=lin[:, h:h + 1], s1=1.0,
                            accum_out=lgj[:, h:h + 1])
                    nc.scalar.activation(expb_all[:, j, :], lgj[:], AF.Exp)

                    # u[dst, h*256:...] += mjh_h^T @ h_em (one fused mult)
                    mjh = m3pool.tile([128, HEADS, 128], BF16, tag="mjh")
                    nc.vector.tensor_tensor(
                        mjh[:],
                        eq[:].rearrange("p (o e) -> p o e", o=1
                                        ).broadcast_to([128, HEADS, 128]),
                        expb_all[:, j, :].rearrange("p (h o) -> p h o", o=1
                                                    ).broadcast_to(
                                                        [128, HEADS, 128]),
                        ALU.mult)
                    for h in range(HEADS):
                        bank = h // 2
                        mm = nc.tensor.matmul(
                            ps_u[:, h * OUTC:(h + 1) * OUTC], mjh[:, h, :],
                            h_em[:],
                            start=(j == 0 and h % 2 == 0),
                            stop=(j == CHUNKS - 1 and h % 2 == 1),
                            skip_group_check=True)
                        if j == 0 and h % 2 == 0:
                            u_first[bank] = mm
                        elif j == 0 and h % 2 == 1:
                            tile.add_dep_helper(
                                mm.ins, u_first[bank].ins,
                                reason="psum zero-region order")
                        elif j == CHUNKS - 1 and h % 2 == 0:
                            u_first[10 + bank] = mm
                        elif j == CHUNKS - 1 and h % 2 == 1:
                            tile.add_dep_helper(
                                mm.ins, u_first[10 + bank].ins,
                                reason="psum stop order")

                # denominators: dt[dst, h] = sum_j eq_j^T @ expb_j
                ps_dt = ps_ld_p.tile([128, 16], F32, tag="ld")
                for j in range(CHUNKS):
                    eqj = m3pool.tile([128, 128], BF16, tag="eq2")
                    nc.vector.tensor_scalar(
                        eqj[:], iota_s[:], dl_t[:, j:j + 1], None,
                        op0=ALU.is_equal)
                    nc.tensor.matmul(
                        ps_dt[:, 0:HEADS], eqj[:], expb_all[:, j, :],
                        start=(j == 0), stop=(j == CHUNKS - 1))
                den = mpool.tile([128, HEADS], F32, tag="den")
                nc.vector.tensor_scalar(
                    den[:], ps_dt[:, 0:HEADS], 1e-30, None, op0=ALU.max)
                nc.vector.reciprocal(rec[:], den[:])

                aggs = mpool.tile([128, 4, 256], BF16, tag="aggs")
                for h in range(HEADS):
                    nc.scalar.activation(
                        aggs[:, h, :], ps_u[:, h * OUTC:(h + 1) * OUTC],
                        AF.Copy, scale=rec[:, h:h + 1])
                return aggs

            def stage_b(aggs, t):
                # uT blocks: [128 hid, (h,half), dst]
                ps_at = ps_at_p.tile([128, 8, 128], BF16, tag="at")
                first = None
                for g in range(8):
                    h, half = g // 2, g % 2
                    mm = nc.tensor.matmul(
                        ps_at[:, g, :],
                        aggs[:, h, half * 128:(half + 1) * 128], i128_s[:],
                        is_transpose=True, start=(g == 0), stop=(g == 7),
                        skip_group_check=True)
                    if g == 0:
                        first = mm
                    else:
                        tile.add_dep_helper(
                            mm.ins, first.ins, reason="psum zero order")
                aggT = mpool.tile([128, 8, 128], BF16, tag="aggT")
                nc.vector.tensor_copy(
                    aggT[:].rearrange("p a b -> p (a b)"),
                    ps_at[:].rearrange("p a b -> p (a b)"))

                # z1T [z1ch-half, dst] = sum_g w_ld1_g^T(co) @ aggT_g
                ps_d1 = ps_d1_p.tile([128, 2, 128], F32, tag="d1")
                for co in range(2):
                    for g in range(8):
                        nc.tensor.matmul(
                            ps_d1[:, co, :],
                            w_ld1_s[:, g, co * 128:(co + 1) * 128],
                            aggT[:, g, :],
                            start=(g == 0), stop=(g == 7),
                            skip_group_check=True)
                d1 = mpool.tile([128, 2, 128], BF16, tag="d1s")
                for co in range(2):
                    nc.scalar.activation(
                        d1[:, co, :], ps_d1[:, co, :], AF.Relu,
                        bias=b_d1p_s[:, co, 0:1])

                ps_o = ps_d1_p.tile([128, 2, 128], F32, tag="d1")
                for co in range(2):
                    nc.tensor.matmul(
                        ps_o[:, 0, 0:6], d1[:, co, :], w_d2_s[:, co, :],
                        start=(co == 0), stop=(co == 1))
                o_sb = mpool.tile([128, 6], F32, tag="o")
                if has_bd2:
                    tmp_o = mpool.tile([128, 6], F32, tag="o2")
                    nc.vector.tensor_tensor(
                        tmp_o[:], ps_o[:, 0, 0:6], b_d2b_s[:], ALU.add)
                    ot = mpool.tile([128, 6], F32, tag="ot")
                    nc.scalar.activation(ot[:], tmp_o[:], AF.Tanh, scale=0.5)
                else:
                    ot = mpool.tile([128, 6], F32, tag="ot")
                    nc.scalar.activation(
                        ot[:], ps_o[:, 0, 0:6], AF.Tanh, scale=0.5)
                # sigmoid(x) = 0.5 + 0.5*tanh(x/2)
                nc.vector.tensor_scalar(
                    o_sb[:], ot[:], 0.5, 0.5, op0=ALU.mult, op1=ALU.add)
                nc.sync.dma_start(out_d[t], o_sb[:])

            pending = None
            for t in range(T):
                ag = stage_a(t)
                if pending is not None:
                    stage_b(*pending)
                pending = (ag, t)
            if pending is not None:
                stage_b(*pending)

    nc.compile()
    return nc


# --------------------------------------------------------------------------
# entry point
# --------------------------------------------------------------------------

def kernel(**inputs):
    in_maps, meta, flags, (deg, iso_out) = _host_prep(inputs)
    if flags not in _CACHE:
        _CACHE[flags] = _build(flags)
    nc = _CACHE[flags]

    from concourse.bass_utils import run_bass_kernel_spmd
    res = run_bass_kernel_spmd(
        nc, in_maps, core_ids=list(range(N_CORES)),
        trace=os.environ.get("BASS_KERNEL_TRACE", "0") == "1")
    kernel.last_exec_time_ns = res.exec_time_ns
    kernel.last_res = res

    out = np.zeros((N_NODES, 6), np.float32)
    for c in range(N_CORES):
        stage = res.results[c]["out"]  # [T, 128, 6]
        for t, (n0, n1) in enumerate(meta[c]):
            if n1 > n0:
                out[n0:n1] = stage[t, :n1 - n0, :]
    iso = deg == 0
    if iso.any():
        out[iso] = iso_out[None, :]
    return out


kernel.last_exec_time_ns = None
